# revision 36
# baseline (speedup 1.0000x reference)
"""Trainium2 Bass kernel for LocalContextEncoder.

out, attn = f(Q, K, V, mask, W1, b1, W2, b2):
  scores = Q @ K^T / sqrt(D)                      [B, L, L]
  conflict[b,i,j] = W2 . relu(Q_i@W1[:D] + K_j@W1[D:] + b1) + b2
  gated = scores * sigmoid(conflict), masked
  attn = softmax_j(gated); out = attn @ V

Sharding: 8 cores, core c -> batch b=c//2, query-row half c%2 (128 rows each).
K/V replicated per batch. Everything on device except weight re-layout.
"""

import sys

for _p in ("/opt/trn_rl_repo", "/opt/trn_rl_repo/concourse"):
    if _p not in sys.path:
        sys.path.insert(0, _p)

import numpy as np
from contextlib import ExitStack

import mybir
from concourse import bass, bacc, tile
from concourse.bass_utils import run_bass_kernel_spmd

B, L, D = 4, 256, 768
NCORES = 8
LI = (B * L) // NCORES  # 128 query rows per core
NCH = D // 128          # 6 contraction chunks

F32 = mybir.dt.float32
F32R = mybir.dt.float32r
BF16 = mybir.dt.bfloat16
I32 = mybir.dt.int32

AF = mybir.ActivationFunctionType
ALU = mybir.AluOpType
AX = mybir.AxisListType

NEG_BIG = -1.0e30


def _build_program():
    nc = bacc.Bacc(
        "TRN2",
        target_bir_lowering=False,
        debug=False,
        enable_asserts=True,
        num_devices=NCORES,
    )

    q_ext = nc.declare_dram_parameter("q", [LI, D], F32, isOutput=False)
    k_ext = nc.declare_dram_parameter("k", [L, D], F32, isOutput=False)
    v_ext = nc.declare_dram_parameter("v", [L, D], F32, isOutput=False)
    m_ext = nc.declare_dram_parameter("msk", [LI, L], I32, isOutput=False)
    w1_ext = nc.declare_dram_parameter("w1", [D, D], BF16, isOutput=False)
    w1k_ext = nc.declare_dram_parameter("w1k", [D, D], BF16, isOutput=False)
    w2b_ext = nc.declare_dram_parameter("w2b", [128, NCH * 64 + 128], BF16, isOutput=False)
    b1_ext = nc.declare_dram_parameter("b1m", [128, NCH], F32, isOutput=False)
    b2_ext = nc.declare_dram_parameter("b2c", [128, 1], F32, isOutput=False)
    id_ext = nc.declare_dram_parameter("ident", [128, 128], F32, isOutput=False)
    o_ext = nc.declare_dram_parameter("o", [LI, D], F32, isOutput=True)
    aw_ext = nc.declare_dram_parameter("aw", [LI, L], F32, isOutput=True)

    with tile.TileContext(nc) as tc:
        with ExitStack() as ctx:
            const = ctx.enter_context(tc.tile_pool(name="const", bufs=1))
            rpv = ctx.enter_context(tc.tile_pool(name="rpv", bufs=8))
            rpa = ctx.enter_context(tc.tile_pool(name="rpa", bufs=4))
            rpg = ctx.enter_context(tc.tile_pool(name="rpg", bufs=3))
            ep = ctx.enter_context(tc.tile_pool(name="ep", bufs=1))
            psA = ctx.enter_context(tc.tile_pool(name="psA", bufs=3, space="PSUM"))
            pcp = ctx.enter_context(tc.tile_pool(name="pcp", bufs=1, space="PSUM"))
            pe2 = ctx.enter_context(tc.tile_pool(name="pe2", bufs=2, space="PSUM"))

            # ---------- persistent SBUF tiles ----------
            ident = const.tile([128, 128], F32, name="ident", tag="ident")
            q_sb = const.tile([128, D], F32, name="q_sb", tag="q_sb")
            k_sb = [const.tile([128, D], F32, name=f"k_sb{j}", tag=f"k_sb{j}") for j in range(2)]
            v_sb = [const.tile([128, D], F32, name=f"v_sb{j}", tag=f"v_sb{j}") for j in range(2)]
            msk_sb = const.tile([128, L], I32, name="msk_sb", tag="msk_sb")
            b1m = const.tile([128, NCH], F32, name="b1m", tag="b1m")
            b2c = const.tile([128, 1], F32, name="b2c", tag="b2c")
            # last 128 cols are zeros: used by the PSUM-clearing dummy matmul
            w2b = const.tile([128, NCH * 64 + 128], BF16, name="w2b", tag="w2b")
            w1t = [
                [const.tile([128, 128], BF16, name=f"w1t_{r}_{m}", tag=f"w1t_{r}_{m}") for m in range(NCH)]
                for r in range(NCH)
            ]
            qtb = [const.tile([128, 128], BF16, name=f"qtb{r}", tag=f"qtb{r}") for r in range(NCH)]
            w1kt = [
                [const.tile([128, 128], BF16, name=f"w1kt_{r}_{m}", tag=f"w1kt_{r}_{m}") for m in range(NCH)]
                for r in range(NCH)
            ]
            qt = [const.tile([128, 128], F32, name=f"qt{r}", tag=f"qt{r}") for r in range(NCH)]
            kt = [const.tile([128, 256], F32, name=f"kt{r}", tag=f"kt{r}") for r in range(NCH)]
            ktb = [const.tile([128, 256], BF16, name=f"ktb{r}", tag=f"ktb{r}") for r in range(NCH)]
            hqt = [const.tile([128, 128], F32, name=f"hqt{m}", tag=f"hqt{m}") for m in range(NCH)]
            hktb = [const.tile([128, 256], BF16, name=f"hktb{m}", tag=f"hktb{m}") for m in range(NCH)]
            scor = const.tile([128, 256], F32, name="scor", tag="scor")
            o_sb = const.tile([128, D], F32, name="o_sb", tag="o_sb")

            # ---------- DMAs (issued up-front; deps gate consumers) ----------
            nc.sync.dma_start(out=ident[:, :], in_=id_ext[:, :])
            # chunked so transposes can start before the whole tensor lands
            for r in range(NCH):
                nc.sync.dma_start(
                    out=q_sb[:, r * 128:(r + 1) * 128],
                    in_=q_ext[:, r * 128:(r + 1) * 128],
                )
            for j in range(2):
                for r in range(NCH):
                    nc.sync.dma_start(
                        out=k_sb[j][:, r * 128:(r + 1) * 128],
                        in_=k_ext[j * 128:(j + 1) * 128, r * 128:(r + 1) * 128],
                    )
            nc.sync.dma_start(out=b1m[:, :], in_=b1_ext[:, :])
            nc.sync.dma_start(out=b2c[:, :], in_=b2_ext[:, :])
            nc.sync.dma_start(out=w2b[:, :], in_=w2b_ext[:, :])
            # W1 tiles, m-major so chunk 0 becomes ready first
            for m in range(NCH):
                for r in range(NCH):
                    nc.sync.dma_start(
                        out=w1t[r][m][:, :],
                        in_=w1_ext[r * 128:(r + 1) * 128, m * 128:(m + 1) * 128],
                    )
                    nc.sync.dma_start(
                        out=w1kt[r][m][:, :],
                        in_=w1k_ext[r * 128:(r + 1) * 128, m * 128:(m + 1) * 128],
                    )
            for j in range(2):
                nc.sync.dma_start(out=v_sb[j][:, :], in_=v_ext[j * 128:(j + 1) * 128, :])
            nc.sync.dma_start(out=msk_sb[:, :], in_=m_ext[:, :])

            # ---------- transposes: qt[r] = Q_chunk^T, kt[r] = K_chunk^T ----------
            for r in range(NCH):
                pt = psA.tile([128, 128], F32, name="psA", tag="psA")
                nc.tensor.transpose(pt[:, :], q_sb[:, r * 128:(r + 1) * 128], ident[:, :])
                nc.vector.tensor_copy(qt[r][:, :], pt[:, :])
                nc.scalar.copy(qtb[r][:, :], pt[:, :])
            for r in range(NCH):
                for j in range(2):
                    pt = psA.tile([128, 128], F32, name="psA", tag="psA")
                    nc.tensor.transpose(
                        pt[:, :], k_sb[j][:, r * 128:(r + 1) * 128], ident[:, :]
                    )
                    if j == 0:
                        nc.scalar.copy(kt[r][:, 0:128], pt[:, :])
                        nc.vector.tensor_copy(ktb[r][:, 0:128], pt[:, :])
                    else:
                        nc.vector.tensor_copy(kt[r][:, 128:256], pt[:, :])
                        nc.scalar.copy(ktb[r][:, 128:256], pt[:, :])

            # ---------- scores = Q K^T / sqrt(D)  (fp32 for accuracy) ----------
            ps_sc = psA.tile([128, 256], F32, name="psA", tag="psA")
            for r in range(NCH):
                nc.tensor.matmul(
                    ps_sc[:, :], qt[r][:, :], kt[r][:, :],
                    start=(r == 0), stop=(r == NCH - 1),
                )
            nc.scalar.mul(scor[:, :], ps_sc[:, :], 1.0 / float(np.sqrt(D)))

            # conflict accumulator (lives across the whole main loop)
            pc = pcp.tile([128, 256], F32, name="pc", tag="pc")

            # mask bias precompute (independent of main loop):
            # mbias = maskf * BIG - BIG  ->  0 where mask==1, -BIG where mask==0
            maskf = ep.tile([128, 256], F32, name="maskf", tag="maskf")
            nc.vector.tensor_copy(maskf[:, :], msk_sb[:, :])
            mbias = ep.tile([128, 256], F32, name="mbias", tag="mbias")
            nc.vector.tensor_scalar(mbias[:, :], maskf[:, :], -NEG_BIG, NEG_BIG, ALU.mult, ALU.add)

            def prologue(m):
                # hqt[m][d, i] = sum_k W1[k, m*128+d] * Q[i, k]
                ph = psA.tile([128, 128], F32, name="psA", tag="psA")
                for r in range(NCH):
                    nc.tensor.matmul(
                        ph[:, :], w1t[r][m][:, :], qtb[r][:, :],
                        start=(r == 0), stop=(r == NCH - 1),
                    )
                nc.vector.tensor_copy(hqt[m][:, :], ph[:, :])
                # hktb[m][d, j] = sum_k W1[D + k, m*128+d] * K[j, k] + b1[m*128+d]
                ph2 = psA.tile([128, 256], F32, name="psA", tag="psA")
                for r in range(NCH):
                    nc.tensor.matmul(
                        ph2[:, :],
                        w1kt[r][m][:, :],
                        ktb[r][:, :],
                        start=(r == 0), stop=(r == NCH - 1),
                    )
                nc.scalar.activation(
                    hktb[m][:, :], ph2[:, :], AF.Identity,
                    bias=b1m[:, m:m + 1], scale=1.0,
                )

            def produce(m, i, eng):
                if eng == "A":
                    rt = rpa.tile([128, 256], BF16, name="ra", tag="ra")
                    nc.scalar.activation(
                        rt[:, :], hktb[m][:, :], AF.Relu,
                        bias=hqt[m][:, i:i + 1], scale=1.0,
                    )
                elif eng == "G":
                    rt = rpg.tile([128, 256], BF16, name="rg", tag="rg")
                    nc.gpsimd.tensor_scalar(
                        rt[:, :], hktb[m][:, :], hqt[m][:, i:i + 1], 0.0,
                        ALU.add, ALU.max,
                    )
                else:
                    rt = rpv.tile([128, 256], BF16, name="rv", tag="rv")
                    nc.vector.tensor_scalar(
                        rt[:, :], hktb[m][:, :], hqt[m][:, i:i + 1], 0.0,
                        ALU.add, ALU.max,
                    )
                return rt

            # engine pattern per (g, q): DVE ~62%, ACT ~38%
            Q3 = ["D", "A", "D", "A"]

            def main_step(m, g):
                # 4 col-groups run concurrently on the PE (tile_position)
                for q in range(4):
                    i = 32 * q + g
                    eng = "D" if q < 2 else ("A" if q == 2 else Q3[g % 4])
                    rt = produce(m, i, eng)
                    last = (m == NCH - 1 and g == 31 and q == 3)
                    s = m * 64 + 32 - g
                    nc.tensor.matmul(
                        pc[32 * q:32 * q + 32, :], w2b[:, s:s + 32], rt[:, :],
                        start=False, stop=last,
                        tile_position=(0, 32 * q),
                        skip_group_check=True,
                    )

            # software-pipelined: prologue(m+1) emitted mid-way through chunk m
            prologue(0)
            # full-width zero matmul: clears has_written + zeroes the whole
            # conflict tile so every col-tiled matmul can accumulate
            nc.tensor.matmul(
                pc[:, :], w2b[:, NCH * 64:NCH * 64 + 128], hktb[0][:, :],
                start=True, stop=False, skip_group_check=True,
            )
            for m in range(NCH):
                for g in range(16):
                    main_step(m, g)
                if m + 1 < NCH:
                    prologue(m + 1)
                for g in range(16, 32):
                    main_step(m, g)

            # ---------- epilogue ----------
            sig = ep.tile([128, 256], F32, name="sig", tag="sig")
            nc.scalar.activation(sig[:, :], pc[:, :], AF.Sigmoid, bias=b2c[:, 0:1], scale=1.0)
            gated = ep.tile([128, 256], F32, name="gated", tag="gated")
            nc.vector.tensor_mul(gated[:, :], sig[:, :], scor[:, :])
            gm = ep.tile([128, 256], F32, name="gm", tag="gm")
            nc.vector.tensor_add(gm[:, :], gated[:, :], mbias[:, :])
            # softmax over free dim
            rmax = ep.tile([128, 1], F32, name="rmax", tag="rmax")
            nc.vector.reduce_max(rmax[:, :], gm[:, :], AX.X)
            nmax = ep.tile([128, 1], F32, name="nmax", tag="nmax")
            nc.vector.tensor_scalar_mul(nmax[:, :], rmax[:, :], -1.0)
            ex = ep.tile([128, 256], F32, name="ex", tag="ex")
            exs = ep.tile([128, 1], F32, name="exs", tag="exs")
            nc.scalar.activation(
                ex[:, :], gm[:, :], AF.Exp,
                bias=nmax[:, 0:1], scale=1.0, accum_out=exs[:, 0:1],
            )
            rcp = ep.tile([128, 1], F32, name="rcp", tag="rcp")
            nc.vector.reciprocal(rcp[:, :], exs[:, :])
            attn = ep.tile([128, 256], F32, name="attn", tag="attn")
            nc.vector.tensor_scalar_mul(attn[:, :], ex[:, :], rcp[:, 0:1])
            nc.sync.dma_start(out=aw_ext[:, :], in_=attn[:, :])

            # out = attn @ V : lhsT = attn^T (two 128-col blocks)
            at = []
            for j in range(2):
                ptj = psA.tile([128, 128], F32, name="psA", tag="psA")
                nc.tensor.transpose(ptj[:, :], attn[:, j * 128:(j + 1) * 128], ident[:, :])
                atj = ep.tile([128, 128], F32, name=f"at{j}", tag=f"at{j}")
                nc.vector.tensor_copy(atj[:, :], ptj[:, :])
                at.append(atj)
            for dh in range(2):
                po = pe2.tile([128, 384], F32, name="po", tag="po")
                for j in range(2):
                    nc.tensor.matmul(
                        po[:, :],
                        at[j][:, :],
                        v_sb[j][:, dh * 384:(dh + 1) * 384],
                        start=(j == 0), stop=(j == 1),
                    )
                if dh == 0:
                    nc.scalar.copy(o_sb[:, 0:384], po[:, :])
                else:
                    nc.vector.tensor_copy(o_sb[:, 384:768], po[:, :])
                nc.sync.dma_start(
                    out=o_ext[:, dh * 384:(dh + 1) * 384],
                    in_=o_sb[:, dh * 384:(dh + 1) * 384],
                )

    nc.finalize()
    return nc


_CACHE = {}


def _get_program():
    if "nc" not in _CACHE:
        _CACHE["nc"] = _build_program()
    return _CACHE["nc"]


def _make_in_maps(Q, K, V, attention_mask, W1, b1, W2, b2):
    Q = np.asarray(Q, np.float32)
    K = np.asarray(K, np.float32)
    V = np.asarray(V, np.float32)
    attention_mask = np.asarray(attention_mask, np.int32)
    W1 = np.asarray(W1, np.float32)
    b1 = np.asarray(b1, np.float32)
    W2 = np.asarray(W2, np.float32)
    b2 = np.asarray(b2, np.float32)

    bf16 = mybir.dt.np(BF16)
    w1q = np.ascontiguousarray(W1[:D]).astype(bf16)
    w1k = np.ascontiguousarray(W1[D:]).astype(bf16)
    w2b = np.zeros((128, NCH * 64 + 128), np.float32)
    for c in range(NCH):
        w2b[:, c * 64 + 32] = W2[c * 128:(c + 1) * 128, 0]
    w2b = w2b.astype(bf16)
    b1m = np.ascontiguousarray(b1.reshape(NCH, 128).T)
    b2c = np.full((128, 1), b2[0], np.float32)
    ident = np.eye(128, dtype=np.float32)

    in_maps = []
    for c in range(NCORES):
        b, half = c // 2, c % 2
        sl = slice(half * LI, (half + 1) * LI)
        in_maps.append({
            "q": np.ascontiguousarray(Q[b, sl]),
            "k": np.ascontiguousarray(K[b]),
            "v": np.ascontiguousarray(V[b]),
            "msk": np.ascontiguousarray(attention_mask[b, sl]),
            "w1": w1q,
            "w1k": w1k,
            "w2b": w2b,
            "b1m": b1m,
            "b2c": b2c,
            "ident": ident,
        })
    return in_maps


def run(Q, K, V, attention_mask, W1, b1, W2, b2, trace=False):
    nc = _get_program()
    in_maps = _make_in_maps(Q, K, V, attention_mask, W1, b1, W2, b2)
    res = run_bass_kernel_spmd(nc, in_maps, list(range(NCORES)), trace=trace)
    out = np.zeros((B, L, D), np.float32)
    attn = np.zeros((B, L, L), np.float32)
    for c in range(NCORES):
        b, half = c // 2, c % 2
        sl = slice(half * LI, (half + 1) * LI)
        out[b, sl] = res.results[c]["o"]
        attn[b, sl] = res.results[c]["aw"]
    return (out, attn), res


def kernel(Q, K, V, attention_mask, W1, b1, W2, b2):
    (out, attn), _ = run(Q, K, V, attention_mask, W1, b1, W2, b2, trace=False)
    return (out, attn)


# revision 43
# speedup vs baseline: 1.1652x; 1.1652x over previous
"""Trainium2 Bass kernel for LocalContextEncoder.

out, attn = f(Q, K, V, mask, W1, b1, W2, b2):
  scores = Q @ K^T / sqrt(D)                      [B, L, L]
  conflict[b,i,j] = W2 . relu(Q_i@W1[:D] + K_j@W1[D:] + b1) + b2
  gated = scores * sigmoid(conflict), masked
  attn = softmax_j(gated); out = attn @ V

Sharding: 8 cores, core c -> batch b=c//2, query-row half c%2 (128 rows each).
K/V replicated per batch. Everything on device except weight re-layout.
"""

import sys

for _p in ("/opt/trn_rl_repo", "/opt/trn_rl_repo/concourse"):
    if _p not in sys.path:
        sys.path.insert(0, _p)

import numpy as np
from contextlib import ExitStack

import mybir
from concourse import bass, bacc, tile
from concourse.bass_utils import run_bass_kernel_spmd

B, L, D = 4, 256, 768
NCORES = 8
LI = (B * L) // NCORES  # 128 query rows per core
NCH = D // 128          # 6 contraction chunks

F32 = mybir.dt.float32
F32R = mybir.dt.float32r
BF16 = mybir.dt.bfloat16
I32 = mybir.dt.int32

AF = mybir.ActivationFunctionType
ALU = mybir.AluOpType
AX = mybir.AxisListType

NEG_BIG = -1.0e30


def _build_program():
    nc = bacc.Bacc(
        "TRN2",
        target_bir_lowering=False,
        debug=False,
        enable_asserts=True,
        num_devices=NCORES,
    )

    q_ext = nc.declare_dram_parameter("q", [LI, D], F32, isOutput=False)
    k_ext = nc.declare_dram_parameter("k", [L, D], F32, isOutput=False)
    v_ext = nc.declare_dram_parameter("v", [L, D], F32, isOutput=False)
    m_ext = nc.declare_dram_parameter("msk", [LI, L], I32, isOutput=False)
    w1_ext = nc.declare_dram_parameter("w1", [D, D], BF16, isOutput=False)
    w1k_ext = nc.declare_dram_parameter("w1k", [D, D], BF16, isOutput=False)
    w2b_ext = nc.declare_dram_parameter("w2b", [128, NCH * 64 + 128], BF16, isOutput=False)
    b1_ext = nc.declare_dram_parameter("b1m", [128, NCH], F32, isOutput=False)
    b2_ext = nc.declare_dram_parameter("b2c", [128, 1], F32, isOutput=False)
    id_ext = nc.declare_dram_parameter("ident", [128, 128], F32, isOutput=False)
    o_ext = nc.declare_dram_parameter("o", [LI, D], F32, isOutput=True)
    aw_ext = nc.declare_dram_parameter("aw", [LI, L], F32, isOutput=True)

    with tile.TileContext(nc) as tc:
        with ExitStack() as ctx:
            const = ctx.enter_context(tc.tile_pool(name="const", bufs=1))
            rpv = ctx.enter_context(tc.tile_pool(name="rpv", bufs=12))
            rpa = ctx.enter_context(tc.tile_pool(name="rpa", bufs=6))
            rpg = ctx.enter_context(tc.tile_pool(name="rpg", bufs=2))
            ep = ctx.enter_context(tc.tile_pool(name="ep", bufs=1))
            psA = ctx.enter_context(tc.tile_pool(name="psA", bufs=3, space="PSUM"))
            pcp = ctx.enter_context(tc.tile_pool(name="pcp", bufs=1, space="PSUM"))
            pe2 = ctx.enter_context(tc.tile_pool(name="pe2", bufs=2, space="PSUM"))

            # ---------- persistent SBUF tiles ----------
            ident = const.tile([128, 128], F32, name="ident", tag="ident")
            q_sb = const.tile([128, D], F32, name="q_sb", tag="q_sb")
            k_sb = [const.tile([128, D], F32, name=f"k_sb{j}", tag=f"k_sb{j}") for j in range(2)]
            v_sb = [const.tile([128, D], F32, name=f"v_sb{j}", tag=f"v_sb{j}") for j in range(2)]
            msk_sb = const.tile([128, L], I32, name="msk_sb", tag="msk_sb")
            b1m = const.tile([128, NCH], F32, name="b1m", tag="b1m")
            b2c = const.tile([128, 1], F32, name="b2c", tag="b2c")
            # last 128 cols are zeros: used by the PSUM-clearing dummy matmul
            w2b = const.tile([128, NCH * 64 + 128], BF16, name="w2b", tag="w2b")
            # w1m[m][p, r*128+c] = W1[r*128+p, m*128+c]  (lhsT tile r at cols r*128)
            w1m = [const.tile([128, D], BF16, name=f"w1m{m}", tag=f"w1m{m}") for m in range(NCH)]
            w1km = [const.tile([128, D], BF16, name=f"w1km{m}", tag=f"w1km{m}") for m in range(NCH)]
            qtb = [const.tile([128, 128], BF16, name=f"qtb{r}", tag=f"qtb{r}") for r in range(NCH)]
            qt = [const.tile([128, 128], F32, name=f"qt{r}", tag=f"qt{r}") for r in range(NCH)]
            kt = [const.tile([128, 256], F32, name=f"kt{r}", tag=f"kt{r}") for r in range(NCH)]
            ktb = [const.tile([128, 256], BF16, name=f"ktb{r}", tag=f"ktb{r}") for r in range(NCH)]
            hqt = [const.tile([128, 128], F32, name=f"hqt{m}", tag=f"hqt{m}") for m in range(NCH)]
            hktb = [const.tile([128, 256], BF16, name=f"hktb{m}", tag=f"hktb{m}") for m in range(NCH)]
            scor = const.tile([128, 256], F32, name="scor", tag="scor")
            o_sb = const.tile([128, D], F32, name="o_sb", tag="o_sb")

            # ---------- DMAs (issued up-front; deps gate consumers) ----------
            nc.sync.dma_start(out=ident[:, :], in_=id_ext[:, :])
            # chunked so transposes can start before the whole tensor lands
            for r in range(NCH):
                nc.sync.dma_start(
                    out=q_sb[:, r * 128:(r + 1) * 128],
                    in_=q_ext[:, r * 128:(r + 1) * 128],
                )
            for j in range(2):
                for r in range(NCH):
                    nc.sync.dma_start(
                        out=k_sb[j][:, r * 128:(r + 1) * 128],
                        in_=k_ext[j * 128:(j + 1) * 128, r * 128:(r + 1) * 128],
                    )
            nc.sync.dma_start(out=b1m[:, :], in_=b1_ext[:, :])
            nc.sync.dma_start(out=b2c[:, :], in_=b2_ext[:, :])
            nc.sync.dma_start(out=w2b[:, :], in_=w2b_ext[:, :])
            # W1 column-blocks, m-major so chunk 0 becomes ready first.
            # One 3D-AP DMA per (m, q/k): DRAM rows (r, p) -> SBUF [p, (r c)]
            for m in range(NCH):
                for ext, dst in ((w1_ext, w1m), (w1k_ext, w1km)):
                    src = ext[:, m * 128:(m + 1) * 128].rearrange(
                        "(r p) c -> p r c", p=128
                    )
                    nc.sync.dma_start(
                        out=dst[m][:, :].rearrange("p (r c) -> p r c", r=NCH),
                        in_=src,
                    )
            for j in range(2):
                nc.sync.dma_start(out=v_sb[j][:, :], in_=v_ext[j * 128:(j + 1) * 128, :])
            nc.sync.dma_start(out=msk_sb[:, :], in_=m_ext[:, :])

            # ---------- transposes: qt[r] = Q_chunk^T, kt[r] = K_chunk^T ----------
            for r in range(NCH):
                pt = psA.tile([128, 128], F32, name="psA", tag="psA")
                nc.tensor.transpose(pt[:, :], q_sb[:, r * 128:(r + 1) * 128], ident[:, :])
                nc.vector.tensor_copy(qt[r][:, :], pt[:, :])
                nc.scalar.copy(qtb[r][:, :], pt[:, :])
            for r in range(NCH):
                for j in range(2):
                    pt = psA.tile([128, 128], F32, name="psA", tag="psA")
                    nc.tensor.transpose(
                        pt[:, :], k_sb[j][:, r * 128:(r + 1) * 128], ident[:, :]
                    )
                    if j == 0:
                        nc.scalar.copy(kt[r][:, 0:128], pt[:, :])
                        nc.vector.tensor_copy(ktb[r][:, 0:128], pt[:, :])
                    else:
                        nc.vector.tensor_copy(kt[r][:, 128:256], pt[:, :])
                        nc.scalar.copy(ktb[r][:, 128:256], pt[:, :])

            # ---------- scores = Q K^T / sqrt(D)  (fp32 for accuracy) ----------
            ps_sc = psA.tile([128, 256], F32, name="psA", tag="psA")
            for r in range(NCH):
                nc.tensor.matmul(
                    ps_sc[:, :], qt[r][:, :], kt[r][:, :],
                    start=(r == 0), stop=(r == NCH - 1),
                )
            nc.scalar.mul(scor[:, :], ps_sc[:, :], 1.0 / float(np.sqrt(D)))

            # conflict accumulator (lives across the whole main loop)
            pc = pcp.tile([128, 256], F32, name="pc", tag="pc")

            # mask bias precompute (independent of main loop):
            # mbias = maskf * BIG - BIG  ->  0 where mask==1, -BIG where mask==0
            maskf = ep.tile([128, 256], F32, name="maskf", tag="maskf")
            nc.vector.tensor_copy(maskf[:, :], msk_sb[:, :])
            mbias = ep.tile([128, 256], F32, name="mbias", tag="mbias")
            nc.vector.tensor_scalar(mbias[:, :], maskf[:, :], -NEG_BIG, NEG_BIG, ALU.mult, ALU.add)

            def prologue(m):
                # hqt[m][d, i] = sum_k W1[k, m*128+d] * Q[i, k]
                ph = psA.tile([128, 128], F32, name="psA", tag="psA")
                for r in range(NCH):
                    nc.tensor.matmul(
                        ph[:, :], w1m[m][:, r * 128:(r + 1) * 128], qtb[r][:, :],
                        start=(r == 0), stop=(r == NCH - 1),
                    )
                nc.vector.tensor_copy(hqt[m][:, :], ph[:, :])
                # hktb[m][d, j] = sum_k W1[D + k, m*128+d] * K[j, k] + b1[m*128+d]
                ph2 = psA.tile([128, 256], F32, name="psA", tag="psA")
                for r in range(NCH):
                    nc.tensor.matmul(
                        ph2[:, :],
                        w1km[m][:, r * 128:(r + 1) * 128],
                        ktb[r][:, :],
                        start=(r == 0), stop=(r == NCH - 1),
                    )
                nc.scalar.activation(
                    hktb[m][:, :], ph2[:, :], AF.Identity,
                    bias=b1m[:, m:m + 1], scale=1.0,
                )

            def produce(m, i, eng):
                if eng == "A":
                    rt = rpa.tile([128, 256], BF16, name="ra", tag="ra")
                    nc.scalar.activation(
                        rt[:, :], hktb[m][:, :], AF.Relu,
                        bias=hqt[m][:, i:i + 1], scale=1.0,
                    )
                elif eng == "G":
                    rt = rpg.tile([128, 256], BF16, name="rg", tag="rg")
                    nc.gpsimd.tensor_scalar(
                        rt[:, :], hktb[m][:, :], hqt[m][:, i:i + 1], 0.0,
                        ALU.add, ALU.max,
                    )
                else:
                    rt = rpv.tile([128, 256], BF16, name="rv", tag="rv")
                    nc.vector.tensor_scalar(
                        rt[:, :], hktb[m][:, :], hqt[m][:, i:i + 1], 0.0,
                        ALU.add, ALU.max,
                    )
                return rt

            # engine pattern per (g, q): DVE ~69%, ACT ~31%
            Q3 = ["D", "A", "D", "D"]

            def main_step(m, g):
                # 4 col-groups run concurrently on the PE (tile_position)
                for q in range(4):
                    i = 32 * q + g
                    eng = "D" if q < 2 else ("A" if q == 2 else Q3[g % 4])
                    rt = produce(m, i, eng)
                    last = (m == NCH - 1 and g == 31 and q == 3)
                    s = m * 64 + 32 - g
                    nc.tensor.matmul(
                        pc[32 * q:32 * q + 32, :], w2b[:, s:s + 32], rt[:, :],
                        start=False, stop=last,
                        tile_position=(0, 32 * q),
                        skip_group_check=True,
                    )

            # software-pipelined: prologue(m+1) emitted mid-way through chunk m
            prologue(0)
            # full-width zero matmul: clears has_written + zeroes the whole
            # conflict tile so every col-tiled matmul can accumulate
            nc.tensor.matmul(
                pc[:, :], w2b[:, NCH * 64:NCH * 64 + 128], hktb[0][:, :],
                start=True, stop=False, skip_group_check=True,
            )
            for m in range(NCH):
                for g in range(16):
                    main_step(m, g)
                if m + 1 < NCH:
                    prologue(m + 1)
                for g in range(16, 32):
                    main_step(m, g)

            # ---------- epilogue ----------
            sig = ep.tile([128, 256], F32, name="sig", tag="sig")
            nc.scalar.activation(sig[:, :], pc[:, :], AF.Sigmoid, bias=b2c[:, 0:1], scale=1.0)
            gated = ep.tile([128, 256], F32, name="gated", tag="gated")
            nc.vector.tensor_mul(gated[:, :], sig[:, :], scor[:, :])
            gm = ep.tile([128, 256], F32, name="gm", tag="gm")
            nc.vector.tensor_add(gm[:, :], gated[:, :], mbias[:, :])
            # softmax over free dim; |gated| is small so exp needs no
            # max-subtraction (masked entries are -1e30 -> exp -> 0)
            ex = ep.tile([128, 256], F32, name="ex", tag="ex")
            exs = ep.tile([128, 1], F32, name="exs", tag="exs")
            nc.scalar.activation(
                ex[:, :], gm[:, :], AF.Exp,
                bias=0.0, scale=1.0, accum_out=exs[:, 0:1],
            )
            rcp = ep.tile([128, 1], F32, name="rcp", tag="rcp")
            nc.vector.reciprocal(rcp[:, :], exs[:, :])
            attn = ep.tile([128, 256], F32, name="attn", tag="attn")
            nc.vector.tensor_scalar_mul(attn[:, :], ex[:, :], rcp[:, 0:1])
            nc.sync.dma_start(out=aw_ext[:, :], in_=attn[:, :])

            # out = attn @ V : lhsT = attn^T (two 128-col blocks)
            at = []
            for j in range(2):
                ptj = psA.tile([128, 128], F32, name="psA", tag="psA")
                nc.tensor.transpose(ptj[:, :], attn[:, j * 128:(j + 1) * 128], ident[:, :])
                atj = ep.tile([128, 128], F32, name=f"at{j}", tag=f"at{j}")
                nc.vector.tensor_copy(atj[:, :], ptj[:, :])
                at.append(atj)
            for dh in range(2):
                po = pe2.tile([128, 384], F32, name="po", tag="po")
                for j in range(2):
                    nc.tensor.matmul(
                        po[:, :],
                        at[j][:, :],
                        v_sb[j][:, dh * 384:(dh + 1) * 384],
                        start=(j == 0), stop=(j == 1),
                    )
                if dh == 0:
                    nc.scalar.copy(o_sb[:, 0:384], po[:, :])
                else:
                    nc.vector.tensor_copy(o_sb[:, 384:768], po[:, :])
                nc.sync.dma_start(
                    out=o_ext[:, dh * 384:(dh + 1) * 384],
                    in_=o_sb[:, dh * 384:(dh + 1) * 384],
                )

    nc.finalize()
    return nc


_CACHE = {}


def _get_program():
    if "nc" not in _CACHE:
        _CACHE["nc"] = _build_program()
    return _CACHE["nc"]


def _make_in_maps(Q, K, V, attention_mask, W1, b1, W2, b2):
    Q = np.asarray(Q, np.float32)
    K = np.asarray(K, np.float32)
    V = np.asarray(V, np.float32)
    attention_mask = np.asarray(attention_mask, np.int32)
    W1 = np.asarray(W1, np.float32)
    b1 = np.asarray(b1, np.float32)
    W2 = np.asarray(W2, np.float32)
    b2 = np.asarray(b2, np.float32)

    bf16 = mybir.dt.np(BF16)
    w1q = np.ascontiguousarray(W1[:D]).astype(bf16)
    w1k = np.ascontiguousarray(W1[D:]).astype(bf16)
    w2b = np.zeros((128, NCH * 64 + 128), np.float32)
    for c in range(NCH):
        w2b[:, c * 64 + 32] = W2[c * 128:(c + 1) * 128, 0]
    w2b = w2b.astype(bf16)
    b1m = np.ascontiguousarray(b1.reshape(NCH, 128).T)
    b2c = np.full((128, 1), b2[0], np.float32)
    ident = np.eye(128, dtype=np.float32)

    in_maps = []
    for c in range(NCORES):
        b, half = c // 2, c % 2
        sl = slice(half * LI, (half + 1) * LI)
        in_maps.append({
            "q": np.ascontiguousarray(Q[b, sl]),
            "k": np.ascontiguousarray(K[b]),
            "v": np.ascontiguousarray(V[b]),
            "msk": np.ascontiguousarray(attention_mask[b, sl]),
            "w1": w1q,
            "w1k": w1k,
            "w2b": w2b,
            "b1m": b1m,
            "b2c": b2c,
            "ident": ident,
        })
    return in_maps


def run(Q, K, V, attention_mask, W1, b1, W2, b2, trace=False):
    nc = _get_program()
    in_maps = _make_in_maps(Q, K, V, attention_mask, W1, b1, W2, b2)
    res = run_bass_kernel_spmd(nc, in_maps, list(range(NCORES)), trace=trace)
    out = np.zeros((B, L, D), np.float32)
    attn = np.zeros((B, L, L), np.float32)
    for c in range(NCORES):
        b, half = c // 2, c % 2
        sl = slice(half * LI, (half + 1) * LI)
        out[b, sl] = res.results[c]["o"]
        attn[b, sl] = res.results[c]["aw"]
    return (out, attn), res


def kernel(Q, K, V, attention_mask, W1, b1, W2, b2):
    (out, attn), _ = run(Q, K, V, attention_mask, W1, b1, W2, b2, trace=False)
    return (out, attn)
